# revision 9
# baseline (speedup 1.0000x reference)
"""Trainium2 Bass kernel for NeuronInvariantDeepSetLayer (segment_reduce).

kernel(**inputs) takes FULL unsharded inputs (as in reference.setup_inputs())
and returns the full [4096, 1] float32 output.

Strategy: data-parallel over 8 NeuronCores. Segments split 512/core (idx is
sorted, so each core's rows are a contiguous slice of x). Rows host-padded so
each 128-segment block starts at a 128-row tile boundary -> identical
instruction stream on every core (pure SPMD).

Key points vs the v1 kernel:
  - x is cast to fp8(e4m3) and pre-TRANSPOSED on the host into the exact
    SBUF layout the PE wants (feature dim on partitions, DoubleRow k-pair
    interleave). HBM traffic for x drops 4x vs f32; no PE transposes.
  - mm1 (x @ W1) runs as fp8 DoubleRow matmuls (K=256/pass, 2 MACs/cycle),
    with W1 pre-scaled by 32 into fp8's sweet spot; relu is positively
    homogeneous so the scale is folded into W2 afterwards.
  - W2 is applied AFTER the segment sum (segsum is linear):
      segsum(relu(x@W1+b1) @ W2 + b2) = segsum(relu(x@W1+b1)) @ W2 + cnt*b2
    so mm2 shrinks from 400k rows to 4096 rows.
  - segment reduce: sel = is_equal(idx_local, iota) one-hot [128 rows, 128
    segs]; matmul(pseg += sel.T @ relu_h) accumulated in PSUM over ~100
    tiles per 128-segment block.
  - per 128-seg block tail (f32): transpose s1, mm2 (+counts*b2), rho.
"""

import sys

sys.path.insert(0, "/opt/trn_rl_repo")

import hashlib

import numpy as np
import ml_dtypes

N = 400000
B = 4096
DIN = 768
DHID = 192
NCORES = 8
SPC = B // NCORES  # segments per core = 512
SBLK = 128  # segments per seg-block (psum accumulator partitions)
NBLK = SPC // SBLK  # 4 seg-blocks per core
P = 128
KJ = DIN // 256  # 3 DoubleRow k-pairs
CH = 2048  # rows per DMA chunk (16 subtiles) -> 12KB/partition lines

f32 = np.float32
bf16 = ml_dtypes.bfloat16
fp8 = ml_dtypes.float8_e4m3

FP8_SCALE = 16.0  # W1 pre-scale into fp8 normal range; undone inside W2


def _prep(x, idx):
    """Host-side sharding + fp8/transpose packing. Layout transforms only."""
    if np.any(np.diff(idx) < 0):  # defensive: spec says idx is sorted
        order = np.argsort(idx, kind="stable")
        x, idx = x[order], idx[order]
    counts = np.bincount(idx, minlength=B)
    assert counts.sum() == x.shape[0]
    bounds = np.concatenate([[0], np.cumsum(counts)]).astype(np.int64)
    blk_rows = counts.reshape(NCORES * NBLK, SBLK).sum(1)
    tblk = int(np.ceil(blk_rows.max() / P))
    tblk = ((tblk + 3) // 4) * 4  # multiple of 4 -> NP % 2048 == 0
    NP = NBLK * tblk * P
    nchunks = NP // CH
    nsub = CH // P
    x8s = np.zeros((NCORES, P, nchunks, KJ, nsub, P, 2), fp8)
    ixs_arr = np.zeros((NCORES, nchunks, P, CH // P, 1), f32)
    cnts = np.zeros((NCORES, 1, SPC), f32)
    ix_pad = np.float32(1.0e9)
    for c in range(NCORES):
        xs = np.zeros((NP, DIN), fp8)
        ixs = np.full(NP, ix_pad, f32)
        for blk in range(NBLK):
            s0 = c * SPC + blk * SBLK
            r0, r1 = int(bounds[s0]), int(bounds[s0 + SBLK])
            d0 = blk * tblk * P
            xs[d0 : d0 + (r1 - r0)] = x[r0:r1].astype(fp8)
            ixs[d0 : d0 + (r1 - r0)] = (idx[r0:r1] - c * SPC).astype(f32)
        # SwInterleave stationary layout: pairs (i) adjacent per column,
        # columns (m = row within subtile) stored in REVERSE order.
        # x8s[c][p, ch, j, s, c_, i] = xs[ch*CH + s*128 + (127-c_), (2j+i)*128 + p]
        y = xs.reshape(nchunks, nsub, P, KJ, 2, P)[:, :, ::-1]
        x8s[c] = y.transpose(5, 0, 3, 1, 2, 4)
        # ixs_arr[c][ch, p, n] = ixs[ch*CH + n*P + p]
        ixs_arr[c] = ixs.reshape(nchunks, CH // P, P).transpose(0, 2, 1)[..., None]
        cnts[c, 0] = counts[c * SPC : (c + 1) * SPC].astype(f32)
    return x8s, ixs_arr, cnts, tblk, counts


def _build(tblk, phi_w1, phi_b1, phi_w2, phi_b2, rho_w1, rho_b1, rho_w2, rho_b2):
    import concourse.bacc as bacc
    import concourse.mybir as mybir
    import concourse.tile as tile

    BF = mybir.dt.bfloat16
    F32 = mybir.dt.float32
    FP8 = mybir.dt.float8e4
    DR = mybir.MatmulPerfMode.DoubleRow
    DRSWI = mybir.MatmulPerfMode.DoubleRowSwInterleave
    Relu = mybir.ActivationFunctionType.Relu
    Copy = mybir.ActivationFunctionType.Copy

    has_b1 = bool(np.any(phi_b1 != 0))
    has_b2 = bool(np.any(phi_b2 != 0))
    has_rb1 = bool(np.any(rho_b1 != 0))
    has_rb2 = bool(np.any(rho_b2 != 0))

    NP = NBLK * tblk * P
    nchunks = NP // CH
    nsub = CH // P  # 16 subtiles per chunk

    # ---- packed constants (inlined into the NEFF) ----
    # w18[p, j, h, i] = W1[(2j+i)*128 + p, h] * FP8_SCALE, fp8 (pair-adjacent)
    w18 = np.ascontiguousarray(
        (phi_w1 * FP8_SCALE).reshape(KJ, 2, P, DHID).transpose(2, 0, 3, 1)
    ).astype(fp8)
    # w2k[p, h1c, h2] = W2[h1c*96 + p, h2] / 32, f32
    w2k = np.ascontiguousarray(
        (phi_w2 / FP8_SCALE).reshape(2, 96, DHID).transpose(1, 0, 2)
    ).astype(f32)
    rw1k = np.ascontiguousarray(rho_w1.reshape(2, 96, 6).transpose(1, 0, 2)).astype(f32)
    rw2k = np.ascontiguousarray(rho_w2).astype(f32)  # [6, 1]
    idn32 = np.eye(P, dtype=f32)
    jmat = np.ascontiguousarray(
        np.broadcast_to(
            (np.arange(NBLK)[:, None, None] * SBLK + np.arange(SBLK)[None, None, :]).astype(f32),
            (P, NBLK, 2, SBLK),
        )
    )
    onesk = np.ones((1, P), bf16)
    b1k = np.ascontiguousarray((phi_b1 * FP8_SCALE).reshape(1, DHID)).astype(bf16)
    b2k = np.ascontiguousarray(phi_b2.reshape(1, DHID)).astype(f32)
    rb1k = np.ascontiguousarray(rho_b1.reshape(6, 1)).astype(f32)
    rb2k = np.ascontiguousarray(rho_b2.reshape(1, 1)).astype(f32)

    nc = bacc.Bacc(None, target_bir_lowering=False)
    x_in = nc.dram_tensor("x8", [P, nchunks, KJ, nsub, P, 2], FP8, kind="ExternalInput")
    ix_in = nc.dram_tensor("idxlf", [nchunks, P, CH // P, 1], F32, kind="ExternalInput")
    cnt_in = nc.dram_tensor("cnts", [1, SPC], F32, kind="ExternalInput")
    out_d = nc.dram_tensor("out_shard", [SPC], F32, kind="ExternalOutput")

    w1d = nc.inline_tensor(w18, "w18")
    w2d = nc.inline_tensor(w2k, "w2k")
    rw1d = nc.inline_tensor(rw1k, "rw1k")
    rw2d = nc.inline_tensor(rw2k, "rw2k")
    idn32d = nc.inline_tensor(idn32, "idn32")
    jmatd = nc.inline_tensor(jmat, "jmat")
    onesd = nc.inline_tensor(onesk, "onesk") if has_b1 else None
    b1d = nc.inline_tensor(b1k, "b1k") if has_b1 else None
    b2d = nc.inline_tensor(b2k, "b2k") if has_b2 else None
    rb1d = nc.inline_tensor(rb1k, "rb1k") if has_rb1 else None
    rb2d = nc.inline_tensor(rb2k, "rb2k") if has_rb2 else None

    with tile.TileContext(nc) as tc:
        with (
            tc.tile_pool(name="consts", bufs=1) as cpool,
            tc.tile_pool(name="xb", bufs=4) as xpool,
            tc.tile_pool(name="ixb", bufs=3) as ixpool,
            tc.tile_pool(name="h1b", bufs=6) as h1pool,
            tc.tile_pool(name="selb", bufs=6) as selpool,
            tc.tile_pool(name="rho", bufs=2) as rhopool,
            tc.tile_pool(name="ph1", bufs=4, space="PSUM") as ph1,
            tc.tile_pool(name="pseg", bufs=2, space="PSUM") as pseg,
            tc.tile_pool(name="ptail", bufs=2, space="PSUM") as ptail,
        ):
            w1s = cpool.tile_from(w1d[:])
            w2s = cpool.tile_from(w2d[:])
            rw1s = cpool.tile_from(rw1d[:])
            rw2s = cpool.tile_from(rw2d[:])
            idn32s = cpool.tile_from(idn32d[:])
            js = cpool.tile_from(jmatd[:])
            cnss = cpool.tile_from(cnt_in[:]) if has_b2 else None
            oness = cpool.tile_from(onesd[:]) if has_b1 else None
            b1s = cpool.tile_from(b1d[:]) if has_b1 else None
            b2s = cpool.tile_from(b2d[:]) if has_b2 else None
            rb1s = cpool.tile_from(rb1d[:]) if has_rb1 else None
            rb2s = cpool.tile_from(rb2d[:]) if has_rb2 else None

            pseg_tiles = {}

            def emit_tail(blk, pseg_t):
                # s1 [128 segs, 192] f32 psum -> out[blk*128:(blk+1)*128]
                s1b = rhopool.tile([P, DHID], F32, tag="s1b")
                nc.scalar.copy(out=s1b[:], in_=pseg_t[:])
                ps1T = ptail.tile([96, 2, P], F32, tag="tt", name=f"ps1T_{blk}")
                for hc in range(2):
                    nc.tensor.transpose(
                        out=ps1T[:, hc, :],
                        in_=s1b[:, hc * 96 : (hc + 1) * 96],
                        identity=idn32s[:],
                    )
                s1Tb = rhopool.tile([96, 2, P], F32, tag="s1Tb")
                nc.vector.tensor_copy(out=s1Tb[:], in_=ps1T[:])
                # x_sumT[h2, seg] = sum_h1 W2[h1,h2]/32 * s1T[h1,seg] + b2*cnt
                pxs = ptail.tile([96, 2, P], F32, tag="tt", name=f"pxs_{blk}")
                for h2c in range(2):
                    for h1c in range(2):
                        nc.tensor.matmul(
                            out=pxs[:, h2c, :],
                            lhsT=w2s[:, h1c, h2c * 96 : (h2c + 1) * 96],
                            rhs=s1Tb[:, h1c, :],
                            start=(h1c == 0),
                            stop=(h1c == 1 and not has_b2),
                        )
                    if has_b2:
                        nc.tensor.matmul(
                            out=pxs[:, h2c, :],
                            lhsT=b2s[:, h2c * 96 : (h2c + 1) * 96],
                            rhs=cnss[:, blk * SBLK : (blk + 1) * SBLK],
                            start=False,
                            stop=True,
                        )
                xsTb = rhopool.tile([96, 2, P], F32, tag="xsTb")
                nc.scalar.copy(out=xsTb[:], in_=pxs[:])
                prt = ptail.tile([6, P], F32, tag="tt", name=f"prt_{blk}")
                for h2c in range(2):
                    nc.tensor.matmul(
                        out=prt[:],
                        lhsT=rw1s[:, h2c, :],
                        rhs=xsTb[:, h2c, :],
                        start=(h2c == 0),
                        stop=(h2c == 1),
                    )
                rtb = rhopool.tile([6, P], F32, tag="rtb")
                if has_rb1:
                    nc.scalar.activation(out=rtb[:], in_=prt[:], func=Relu, bias=rb1s[:])
                else:
                    nc.scalar.activation(out=rtb[:], in_=prt[:], func=Relu)
                pot = ptail.tile([1, P], F32, tag="tt", name=f"pot_{blk}")
                nc.tensor.matmul(out=pot[:], lhsT=rw2s[:], rhs=rtb[:], start=True, stop=True)
                ob = rhopool.tile([1, P], F32, tag="ob")
                if has_rb2:
                    nc.scalar.activation(out=ob[:], in_=pot[:], func=Copy, bias=rb2s[:])
                else:
                    nc.scalar.copy(out=ob[:], in_=pot[:])
                nc.sync.dma_start(out=out_d[blk * SBLK : (blk + 1) * SBLK], in_=ob[:])

            pending = None

            def emit_seg(t0, blk, h1b, sel2):
                if t0 % tblk == 0:
                    pseg_tiles[blk] = pseg.tile(
                        [P, DHID], F32, tag="seg", name=f"pseg_{blk}"
                    )
                nc.tensor.matmul(
                    out=pseg_tiles[blk][:],
                    lhsT=sel2[:],
                    rhs=h1b[:],
                    start=(t0 % tblk == 0),
                    stop=(t0 % tblk == tblk - 2),
                    perf_mode=DR,
                )
                if t0 % tblk == tblk - 2:
                    emit_tail(blk, pseg_tiles.pop(blk))

            for ch in range(nchunks):
                xtb = xpool.tile([P, KJ, nsub, P, 2], FP8, tag="xtb")
                if ch == 0:
                    # split first chunk so the PE starts ~4us sooner
                    for q in range(4):
                        nc.gpsimd.dma_start(
                            out=xtb[:, :, q * 4 : (q + 1) * 4],
                            in_=x_in[:, 0, :, q * 4 : (q + 1) * 4],
                        )
                else:
                    nc.gpsimd.dma_start(out=xtb[:], in_=x_in[:, ch])
                ixb = ixpool.tile([P, CH // P, 1], F32, tag="ixb")
                nc.sync.dma_start(out=ixb[:], in_=ix_in[ch])
                # process subtiles in pairs: one [P, 2, DHID] psum tile, a
                # single relu and a single is_equal per pair, and one fp8
                # DoubleRow seg matmul per pair (tblk % 4 == 0, so a pair
                # never straddles a seg-block boundary). Seg matmuls are
                # emitted one pair LATE so the in-order PE never waits on
                # the relu/sel of the pair it just computed.
                for sp in range(nsub // 2):
                    t0 = ch * nsub + sp * 2
                    blk = t0 // tblk
                    ph1t = ph1.tile([P, 2, DHID], F32, tag="h1", name=f"ph1_{t0}")
                    for k in range(2):
                        sub = sp * 2 + k
                        for j in range(KJ):
                            nc.tensor.matmul(
                                out=ph1t[:, k, :],
                                lhsT=xtb[:, j, sub],
                                rhs=w1s[:, j].transpose([0, 2, 1]),
                                start=(j == 0),
                                stop=(j == KJ - 1 and not has_b1),
                                perf_mode=DRSWI,
                            )
                        if has_b1:
                            nc.tensor.matmul(
                                out=ph1t[:, k, :], lhsT=oness[:], rhs=b1s[:],
                                start=False, stop=True,
                            )
                    if pending is not None:
                        emit_seg(*pending)
                    h1b = h1pool.tile([P, 2, DHID], FP8, tag="h1b", name=f"h1b_{t0}")
                    nc.scalar.activation(out=h1b[:], in_=ph1t[:], func=Relu)
                    sel2 = selpool.tile([P, 2, P], FP8, tag="selb", name=f"sel_{t0}")
                    nc.vector.tensor_tensor(
                        out=sel2[:],
                        in0=ixb[:, sp * 2 : sp * 2 + 2, :].to_broadcast([P, 2, P]),
                        in1=js[:, blk, :, :],
                        op=mybir.AluOpType.is_equal,
                    )
                    pending = (t0, blk, h1b, sel2)
            emit_seg(*pending)

    nc.compile()
    return nc


_CACHE = {}


def _get_nc(tblk, weights):
    h = hashlib.md5()
    for w in weights:
        h.update(np.ascontiguousarray(w).tobytes())
    key = (tblk, h.hexdigest())
    if key not in _CACHE:
        _CACHE[key] = _build(tblk, *weights)
    return _CACHE[key]


def _run(inputs, trace=False):
    from concourse.bass_utils import run_bass_kernel_spmd

    inp = {k: np.asarray(v) for k, v in inputs.items()}
    x = inp["x"].astype(f32, copy=False)
    idx = inp["idx"].astype(np.int32, copy=False)
    weights = tuple(
        inp[k].astype(f32, copy=False)
        for k in ("phi_w1", "phi_b1", "phi_w2", "phi_b2", "rho_w1", "rho_b1", "rho_w2", "rho_b2")
    )
    x8s, ixs, cnts, tblk, counts = _prep(x, idx)
    nc = _get_nc(tblk, weights)
    in_maps = [
        {"x8": x8s[c], "idxlf": ixs[c], "cnts": cnts[c]} for c in range(NCORES)
    ]
    res = run_bass_kernel_spmd(nc, in_maps, core_ids=list(range(NCORES)), trace=trace)
    out = np.concatenate([res.results[c]["out_shard"] for c in range(NCORES)])
    out = out.reshape(B, 1).astype(f32)
    return out, res


def kernel(**inputs) -> np.ndarray:
    return _run(inputs, trace=False)[0]


if __name__ == "__main__":
    # quick self-test against numpy
    rng = np.random.default_rng(0)
    x = rng.standard_normal((N, DIN)).astype(f32)
    idx = np.sort(rng.integers(0, B, N).astype(np.int32))
    w1 = (rng.standard_normal((DIN, DHID)) / np.sqrt(DIN)).astype(f32)
    w2 = (rng.standard_normal((DHID, DHID)) / np.sqrt(DHID)).astype(f32)
    r1 = (rng.standard_normal((DHID, 6)) / np.sqrt(DHID)).astype(f32)
    r2 = (rng.standard_normal((6, 1)) / np.sqrt(6)).astype(f32)
    inputs = dict(
        x=x, idx=idx,
        phi_w1=w1, phi_b1=np.zeros(DHID, f32), phi_w2=w2, phi_b2=np.zeros(DHID, f32),
        rho_w1=r1, rho_b1=np.zeros(6, f32), rho_w2=r2, rho_b2=np.zeros(1, f32),
    )
    out = kernel(**inputs)
    h = np.maximum(x @ w1, 0.0) @ w2
    xsum = np.zeros((B, DHID), f32)
    np.add.at(xsum, idx, h)
    exp = np.maximum(xsum @ r1, 0.0) @ r2
    rel = np.linalg.norm(out - exp) / np.linalg.norm(exp)
    print("self-test rel err:", rel)


# revision 10
# speedup vs baseline: 1.0234x; 1.0234x over previous
"""Trainium2 Bass kernel for NeuronInvariantDeepSetLayer (segment_reduce).

kernel(**inputs) takes FULL unsharded inputs (as in reference.setup_inputs())
and returns the full [4096, 1] float32 output.

Strategy: data-parallel over 8 NeuronCores. Segments split 512/core (idx is
sorted, so each core's rows are a contiguous slice of x). Rows host-padded so
each 128-segment block starts at a 128-row tile boundary -> identical
instruction stream on every core (pure SPMD).

Key points vs the v1 kernel:
  - x is cast to fp8(e4m3) and pre-TRANSPOSED on the host into the exact
    SBUF layout the PE wants (feature dim on partitions, DoubleRow k-pair
    interleave). HBM traffic for x drops 4x vs f32; no PE transposes.
  - mm1 (x @ W1) runs as fp8 DoubleRow matmuls (K=256/pass, 2 MACs/cycle),
    with W1 pre-scaled by 32 into fp8's sweet spot; relu is positively
    homogeneous so the scale is folded into W2 afterwards.
  - W2 is applied AFTER the segment sum (segsum is linear):
      segsum(relu(x@W1+b1) @ W2 + b2) = segsum(relu(x@W1+b1)) @ W2 + cnt*b2
    so mm2 shrinks from 400k rows to 4096 rows.
  - segment reduce: sel = is_equal(idx_local, iota) one-hot [128 rows, 128
    segs]; matmul(pseg += sel.T @ relu_h) accumulated in PSUM over ~100
    tiles per 128-segment block.
  - per 128-seg block tail (f32): transpose s1, mm2 (+counts*b2), rho.
"""

import sys

sys.path.insert(0, "/opt/trn_rl_repo")

import hashlib

import numpy as np
import ml_dtypes

N = 400000
B = 4096
DIN = 768
DHID = 192
NCORES = 8
SPC = B // NCORES  # segments per core = 512
SBLK = 128  # segments per seg-block (psum accumulator partitions)
NBLK = SPC // SBLK  # 4 seg-blocks per core
P = 128
KJ = DIN // 256  # 3 DoubleRow k-pairs
CH = 2048  # rows per DMA chunk (16 subtiles) -> 12KB/partition lines

f32 = np.float32
bf16 = ml_dtypes.bfloat16
fp8 = ml_dtypes.float8_e4m3

FP8_SCALE = 16.0  # W1 pre-scale into fp8 normal range; undone inside W2


def _prep(x, idx):
    """Host-side sharding + fp8/transpose packing. Layout transforms only."""
    if np.any(np.diff(idx) < 0):  # defensive: spec says idx is sorted
        order = np.argsort(idx, kind="stable")
        x, idx = x[order], idx[order]
    counts = np.bincount(idx, minlength=B)
    assert counts.sum() == x.shape[0]
    bounds = np.concatenate([[0], np.cumsum(counts)]).astype(np.int64)
    blk_rows = counts.reshape(NCORES * NBLK, SBLK).sum(1)
    tblk = int(np.ceil(blk_rows.max() / P))
    tblk = ((tblk + 3) // 4) * 4  # multiple of 4 -> NP % 2048 == 0
    NP = NBLK * tblk * P
    nchunks = NP // CH
    nsub = CH // P
    x8s = np.zeros((NCORES, P, nchunks, KJ, nsub, P, 2), fp8)
    ixs_arr = np.zeros((NCORES, nchunks, P, CH // P, 1), f32)
    cnts = np.zeros((NCORES, 1, SPC), f32)
    ix_pad = np.float32(1.0e9)
    for c in range(NCORES):
        xs = np.zeros((NP, DIN), fp8)
        ixs = np.full(NP, ix_pad, f32)
        for blk in range(NBLK):
            s0 = c * SPC + blk * SBLK
            r0, r1 = int(bounds[s0]), int(bounds[s0 + SBLK])
            d0 = blk * tblk * P
            xs[d0 : d0 + (r1 - r0)] = x[r0:r1].astype(fp8)
            ixs[d0 : d0 + (r1 - r0)] = (idx[r0:r1] - c * SPC).astype(f32)
        # SwInterleave stationary layout: pairs (i) adjacent per column,
        # columns (m = row within subtile) stored in REVERSE order.
        # x8s[c][p, ch, j, s, c_, i] = xs[ch*CH + s*128 + (127-c_), (2j+i)*128 + p]
        y = xs.reshape(nchunks, nsub, P, KJ, 2, P)[:, :, ::-1]
        x8s[c] = y.transpose(5, 0, 3, 1, 2, 4)
        # ixs_arr[c][ch, p, n] = ixs[ch*CH + n*P + p]
        ixs_arr[c] = ixs.reshape(nchunks, CH // P, P).transpose(0, 2, 1)[..., None]
        cnts[c, 0] = counts[c * SPC : (c + 1) * SPC].astype(f32)
    return x8s, ixs_arr, cnts, tblk, counts


def _build(tblk, phi_w1, phi_b1, phi_w2, phi_b2, rho_w1, rho_b1, rho_w2, rho_b2):
    import concourse.bacc as bacc
    import concourse.mybir as mybir
    import concourse.tile as tile

    BF = mybir.dt.bfloat16
    F32 = mybir.dt.float32
    FP8 = mybir.dt.float8e4
    DR = mybir.MatmulPerfMode.DoubleRow
    DRSWI = mybir.MatmulPerfMode.DoubleRowSwInterleave
    Relu = mybir.ActivationFunctionType.Relu
    Copy = mybir.ActivationFunctionType.Copy

    has_b1 = bool(np.any(phi_b1 != 0))
    has_b2 = bool(np.any(phi_b2 != 0))
    has_rb1 = bool(np.any(rho_b1 != 0))
    has_rb2 = bool(np.any(rho_b2 != 0))

    NP = NBLK * tblk * P
    nchunks = NP // CH
    nsub = CH // P  # 16 subtiles per chunk

    # ---- packed constants (inlined into the NEFF) ----
    # w18[p, j, h, i] = W1[(2j+i)*128 + p, h] * FP8_SCALE, fp8 (pair-adjacent)
    w18 = np.ascontiguousarray(
        (phi_w1 * FP8_SCALE).reshape(KJ, 2, P, DHID).transpose(2, 0, 3, 1)
    ).astype(fp8)
    # w2k[p, h1c, h2] = W2[h1c*96 + p, h2] / 32, f32
    w2k = np.ascontiguousarray(
        (phi_w2 / FP8_SCALE).reshape(2, 96, DHID).transpose(1, 0, 2)
    ).astype(f32)
    rw1k = np.ascontiguousarray(rho_w1.reshape(2, 96, 6).transpose(1, 0, 2)).astype(f32)
    rw2k = np.ascontiguousarray(rho_w2).astype(f32)  # [6, 1]
    idn32 = np.eye(P, dtype=f32)
    jmat = np.ascontiguousarray(
        np.broadcast_to(
            (np.arange(NBLK)[:, None, None] * SBLK + np.arange(SBLK)[None, None, :]).astype(f32),
            (P, NBLK, 4, SBLK),
        )
    )
    onesk = np.ones((1, P), bf16)
    b1k = np.ascontiguousarray((phi_b1 * FP8_SCALE).reshape(1, DHID)).astype(bf16)
    b2k = np.ascontiguousarray(phi_b2.reshape(1, DHID)).astype(f32)
    rb1k = np.ascontiguousarray(rho_b1.reshape(6, 1)).astype(f32)
    rb2k = np.ascontiguousarray(rho_b2.reshape(1, 1)).astype(f32)

    nc = bacc.Bacc(None, target_bir_lowering=False)
    x_in = nc.dram_tensor("x8", [P, nchunks, KJ, nsub, P, 2], FP8, kind="ExternalInput")
    ix_in = nc.dram_tensor("idxlf", [nchunks, P, CH // P, 1], F32, kind="ExternalInput")
    cnt_in = nc.dram_tensor("cnts", [1, SPC], F32, kind="ExternalInput")
    out_d = nc.dram_tensor("out_shard", [SPC], F32, kind="ExternalOutput")

    w1d = nc.inline_tensor(w18, "w18")
    w2d = nc.inline_tensor(w2k, "w2k")
    rw1d = nc.inline_tensor(rw1k, "rw1k")
    rw2d = nc.inline_tensor(rw2k, "rw2k")
    idn32d = nc.inline_tensor(idn32, "idn32")
    jmatd = nc.inline_tensor(jmat, "jmat")
    onesd = nc.inline_tensor(onesk, "onesk") if has_b1 else None
    b1d = nc.inline_tensor(b1k, "b1k") if has_b1 else None
    b2d = nc.inline_tensor(b2k, "b2k") if has_b2 else None
    rb1d = nc.inline_tensor(rb1k, "rb1k") if has_rb1 else None
    rb2d = nc.inline_tensor(rb2k, "rb2k") if has_rb2 else None

    with tile.TileContext(nc) as tc:
        with (
            tc.tile_pool(name="consts", bufs=1) as cpool,
            tc.tile_pool(name="xb", bufs=5) as xpool,
            tc.tile_pool(name="ixb", bufs=3) as ixpool,
            tc.tile_pool(name="h1b", bufs=6) as h1pool,
            tc.tile_pool(name="selb", bufs=6) as selpool,
            tc.tile_pool(name="rho", bufs=2) as rhopool,
            tc.tile_pool(name="ph1", bufs=5, space="PSUM") as ph1,
            tc.tile_pool(name="pseg", bufs=2, space="PSUM") as pseg,
            tc.tile_pool(name="ptail", bufs=1, space="PSUM") as ptail,
        ):
            w1s = cpool.tile_from(w1d[:])
            w2s = cpool.tile_from(w2d[:])
            rw1s = cpool.tile_from(rw1d[:])
            rw2s = cpool.tile_from(rw2d[:])
            idn32s = cpool.tile_from(idn32d[:])
            js = cpool.tile_from(jmatd[:])
            cnss = cpool.tile_from(cnt_in[:]) if has_b2 else None
            oness = cpool.tile_from(onesd[:]) if has_b1 else None
            b1s = cpool.tile_from(b1d[:]) if has_b1 else None
            b2s = cpool.tile_from(b2d[:]) if has_b2 else None
            rb1s = cpool.tile_from(rb1d[:]) if has_rb1 else None
            rb2s = cpool.tile_from(rb2d[:]) if has_rb2 else None

            pseg_tiles = {}

            def emit_tail(blk, pseg_t):
                # s1 [128 segs, 192] f32 psum -> out[blk*128:(blk+1)*128]
                s1b = rhopool.tile([P, DHID], F32, tag="s1b")
                nc.scalar.copy(out=s1b[:], in_=pseg_t[:])
                ps1T = ptail.tile([96, 2, P], F32, tag="tt", name=f"ps1T_{blk}")
                for hc in range(2):
                    nc.tensor.transpose(
                        out=ps1T[:, hc, :],
                        in_=s1b[:, hc * 96 : (hc + 1) * 96],
                        identity=idn32s[:],
                    )
                s1Tb = rhopool.tile([96, 2, P], F32, tag="s1Tb")
                nc.vector.tensor_copy(out=s1Tb[:], in_=ps1T[:])
                # x_sumT[h2, seg] = sum_h1 W2[h1,h2]/32 * s1T[h1,seg] + b2*cnt
                pxs = ptail.tile([96, 2, P], F32, tag="tt", name=f"pxs_{blk}")
                for h2c in range(2):
                    for h1c in range(2):
                        nc.tensor.matmul(
                            out=pxs[:, h2c, :],
                            lhsT=w2s[:, h1c, h2c * 96 : (h2c + 1) * 96],
                            rhs=s1Tb[:, h1c, :],
                            start=(h1c == 0),
                            stop=(h1c == 1 and not has_b2),
                        )
                    if has_b2:
                        nc.tensor.matmul(
                            out=pxs[:, h2c, :],
                            lhsT=b2s[:, h2c * 96 : (h2c + 1) * 96],
                            rhs=cnss[:, blk * SBLK : (blk + 1) * SBLK],
                            start=False,
                            stop=True,
                        )
                xsTb = rhopool.tile([96, 2, P], F32, tag="xsTb")
                nc.scalar.copy(out=xsTb[:], in_=pxs[:])
                prt = ptail.tile([6, P], F32, tag="tt", name=f"prt_{blk}")
                for h2c in range(2):
                    nc.tensor.matmul(
                        out=prt[:],
                        lhsT=rw1s[:, h2c, :],
                        rhs=xsTb[:, h2c, :],
                        start=(h2c == 0),
                        stop=(h2c == 1),
                    )
                rtb = rhopool.tile([6, P], F32, tag="rtb")
                if has_rb1:
                    nc.scalar.activation(out=rtb[:], in_=prt[:], func=Relu, bias=rb1s[:])
                else:
                    nc.scalar.activation(out=rtb[:], in_=prt[:], func=Relu)
                pot = ptail.tile([1, P], F32, tag="tt", name=f"pot_{blk}")
                nc.tensor.matmul(out=pot[:], lhsT=rw2s[:], rhs=rtb[:], start=True, stop=True)
                ob = rhopool.tile([1, P], F32, tag="ob")
                if has_rb2:
                    nc.scalar.activation(out=ob[:], in_=pot[:], func=Copy, bias=rb2s[:])
                else:
                    nc.scalar.copy(out=ob[:], in_=pot[:])
                nc.sync.dma_start(out=out_d[blk * SBLK : (blk + 1) * SBLK], in_=ob[:])

            pending = []

            def emit_seg(t0, blk, h1b, sel2):
                if t0 % tblk == 0:
                    pseg_tiles[blk] = pseg.tile(
                        [P, DHID], F32, tag="seg", name=f"pseg_{blk}"
                    )
                nc.tensor.matmul(
                    out=pseg_tiles[blk][:],
                    lhsT=sel2,
                    rhs=h1b[:],
                    start=(t0 % tblk == 0),
                    stop=(t0 % tblk == tblk - 2),
                    perf_mode=DR,
                )
                if t0 % tblk == tblk - 2:
                    emit_tail(blk, pseg_tiles.pop(blk))

            for ch in range(nchunks):
                xtb = xpool.tile([P, KJ, nsub, P, 2], FP8, tag="xtb")
                if ch == 0:
                    # split first chunk so the PE starts ~4us sooner
                    for q in range(8):
                        nc.gpsimd.dma_start(
                            out=xtb[:, :, q * 2 : (q + 1) * 2],
                            in_=x_in[:, 0, :, q * 2 : (q + 1) * 2],
                        )
                else:
                    nc.gpsimd.dma_start(out=xtb[:], in_=x_in[:, ch])
                ixb = ixpool.tile([P, CH // P, 1], F32, tag="ixb")
                nc.sync.dma_start(out=ixb[:], in_=ix_in[ch])
                # process subtiles in pairs: one [P, 2, DHID] psum tile, a
                # single relu and a single is_equal per pair, and one fp8
                # DoubleRow seg matmul per pair (tblk % 4 == 0, so a pair
                # never straddles a seg-block boundary). Seg matmuls are
                # emitted one pair LATE so the in-order PE never waits on
                # the relu/sel of the pair it just computed.
                for sp in range(nsub // 2):
                    t0 = ch * nsub + sp * 2
                    blk = t0 // tblk
                    ph1t = ph1.tile([P, 2, DHID], F32, tag="h1", name=f"ph1_{t0}")
                    for k in range(2):
                        sub = sp * 2 + k
                        for j in range(KJ):
                            nc.tensor.matmul(
                                out=ph1t[:, k, :],
                                lhsT=xtb[:, j, sub],
                                rhs=w1s[:, j].transpose([0, 2, 1]),
                                start=(j == 0),
                                stop=(j == KJ - 1 and not has_b1),
                                perf_mode=DRSWI,
                            )
                        if has_b1:
                            nc.tensor.matmul(
                                out=ph1t[:, k, :], lhsT=oness[:], rhs=b1s[:],
                                start=False, stop=True,
                            )
                    if len(pending) == 2:
                        emit_seg(*pending.pop(0))
                    h1b = h1pool.tile([P, 2, DHID], FP8, tag="h1b", name=f"h1b_{t0}")
                    nc.scalar.activation(out=h1b[:], in_=ph1t[:], func=Relu)
                    if sp % 2 == 0:
                        sel4 = selpool.tile([P, 4, P], FP8, tag="selb", name=f"sel_{t0}")
                        nc.vector.tensor_tensor(
                            out=sel4[:],
                            in0=ixb[:, sp * 2 : sp * 2 + 4, :].to_broadcast([P, 4, P]),
                            in1=js[:, blk, :, :],
                            op=mybir.AluOpType.is_equal,
                        )
                    pending.append((t0, blk, h1b, sel4[:, (sp % 2) * 2 : (sp % 2) * 2 + 2, :]))
            for pd in pending:
                emit_seg(*pd)
            pending = []

    nc.compile()
    return nc


_CACHE = {}


def _get_nc(tblk, weights):
    h = hashlib.md5()
    for w in weights:
        h.update(np.ascontiguousarray(w).tobytes())
    key = (tblk, h.hexdigest())
    if key not in _CACHE:
        _CACHE[key] = _build(tblk, *weights)
    return _CACHE[key]


def _run(inputs, trace=False):
    from concourse.bass_utils import run_bass_kernel_spmd

    inp = {k: np.asarray(v) for k, v in inputs.items()}
    x = inp["x"].astype(f32, copy=False)
    idx = inp["idx"].astype(np.int32, copy=False)
    weights = tuple(
        inp[k].astype(f32, copy=False)
        for k in ("phi_w1", "phi_b1", "phi_w2", "phi_b2", "rho_w1", "rho_b1", "rho_w2", "rho_b2")
    )
    x8s, ixs, cnts, tblk, counts = _prep(x, idx)
    nc = _get_nc(tblk, weights)
    in_maps = [
        {"x8": x8s[c], "idxlf": ixs[c], "cnts": cnts[c]} for c in range(NCORES)
    ]
    res = run_bass_kernel_spmd(nc, in_maps, core_ids=list(range(NCORES)), trace=trace)
    out = np.concatenate([res.results[c]["out_shard"] for c in range(NCORES)])
    out = out.reshape(B, 1).astype(f32)
    return out, res


def kernel(**inputs) -> np.ndarray:
    return _run(inputs, trace=False)[0]


if __name__ == "__main__":
    # quick self-test against numpy
    rng = np.random.default_rng(0)
    x = rng.standard_normal((N, DIN)).astype(f32)
    idx = np.sort(rng.integers(0, B, N).astype(np.int32))
    w1 = (rng.standard_normal((DIN, DHID)) / np.sqrt(DIN)).astype(f32)
    w2 = (rng.standard_normal((DHID, DHID)) / np.sqrt(DHID)).astype(f32)
    r1 = (rng.standard_normal((DHID, 6)) / np.sqrt(DHID)).astype(f32)
    r2 = (rng.standard_normal((6, 1)) / np.sqrt(6)).astype(f32)
    inputs = dict(
        x=x, idx=idx,
        phi_w1=w1, phi_b1=np.zeros(DHID, f32), phi_w2=w2, phi_b2=np.zeros(DHID, f32),
        rho_w1=r1, rho_b1=np.zeros(6, f32), rho_w2=r2, rho_b2=np.zeros(1, f32),
    )
    out = kernel(**inputs)
    h = np.maximum(x @ w1, 0.0) @ w2
    xsum = np.zeros((B, DHID), f32)
    np.add.at(xsum, idx, h)
    exp = np.maximum(xsum @ r1, 0.0) @ r2
    rel = np.linalg.norm(out - exp) / np.linalg.norm(exp)
    print("self-test rel err:", rel)
